# revision 16
# baseline (speedup 1.0000x reference)
"""Trainium2 Bass kernel for nn_Attention_6313601925220 (sparse_attention).

Reference computation (per (b,h) head; K == Q):
    QR = rope(Q)                      # interleaved-pair RoPE, phases = t * freqs[n]
    scores = tril(QR @ QR^T, k=-1)    # strictly causal, NO softmax
    out = scores @ V

Because there is no softmax, the strictly-causal masked product is linear and
is computed with the chunked linear-attention prefix scan:
    P_i = sum_{j<i} QR_j^T V_j                  # [N, DV] running state (PSUM, f32)
    out_i = QR_i @ P_i + tril_strict(QR_i QR_i^T) @ V_i
This is ~8x fewer FLOPs than the dense T x T score matrix (headroom=8).

Implementation notes:
  - bf16 compute on TensorE (1 cyc/row vs 4 for f32); f32 accumulation in PSUM.
  - RoPE: a = Q*cos and b = pairswap(Q)*signed_sin both run on GPSIMD (the
    pair swap is a reversed-stride access pattern, validated on HW); the add
    alternates DVE/GPSIMD by chunk parity. cos/signed-sin tables come from
    the host (computed from the freqs input).
  - P state accumulates in a persistent PSUM bank per head with a long-open
    accumulation group (HW-validated). start_tensor_calc=True clears
    has_written for the WHOLE 2KB psum bank, so only the first matmul
    touching a multi-region bank may set it.
  - Outputs accumulate 8 chunks per PSUM bank before one big evacuation;
    evacuation work is spread across ScalarE/VectorE by parity.
  - DRAM traffic is staged through SBUF in quarter-head DMA pieces,
    need-ordered so compute starts as soon as the first pieces land.

Sharding: B*NH = 32 heads, 4 heads per core across 8 cores; heads are fully
independent - no collectives.
"""

import os
import math

os.environ.setdefault("MYCRO_LOCAL_CACHE", "1")

import numpy as np
import ml_dtypes

from contextlib import ExitStack

import concourse.bass as bass
import concourse.tile as tile
from concourse import bacc, mybir
from concourse.bass_utils import run_bass_kernel_spmd

# Problem shapes (hardcoded per spec)
B, NH, T, N, DV = 2, 16, 2048, 256, 64
NCORES = 8
BH = B * NH              # 32 heads total
HPC = BH // NCORES       # 4 heads per core
TH = T * HPC             # 8192 rows of (t) per core
CH = 128                 # chunk length along t
NCH = T // CH            # 16 chunks per head

F32 = mybir.dt.float32
BF16 = mybir.dt.bfloat16
NPBF16 = ml_dtypes.bfloat16


def _build_nc():
    nc = bacc.Bacc(None, target_bir_lowering=False)

    q_d = nc.dram_tensor("q", [TH, N], BF16, kind="ExternalInput")
    v_d = nc.dram_tensor("v", [TH, DV], BF16, kind="ExternalInput")
    c_d = nc.dram_tensor("ctab", [T, N], BF16, kind="ExternalInput")   # cos table
    s_d = nc.dram_tensor("stab", [T, N], BF16, kind="ExternalInput")   # signed sin
    o_d = nc.dram_tensor("out", [TH, DV], BF16, kind="ExternalOutput")

    ident_d = nc.inline_tensor(np.eye(128).astype(NPBF16), "ident_c")
    # ST layout is [s, tq]; keep strictly-causal entries s < tq -> strict upper
    mask_d = nc.inline_tensor(np.triu(np.ones((128, 128)), k=1).astype(NPBF16),
                              "mask_c")

    with tile.TileContext(nc) as tc, ExitStack() as ctx:
        consts = ctx.enter_context(tc.tile_pool(name="consts", bufs=1))
        rope = ctx.enter_context(tc.tile_pool(name="rope", bufs=8))
        qrtp = ctx.enter_context(tc.tile_pool(name="qrt", bufs=4))
        stp = ctx.enter_context(tc.tile_pool(name="st", bufs=4))
        pp = ctx.enter_context(tc.tile_pool(name="pst", bufs=10))
        ps_t = ctx.enter_context(tc.tile_pool(name="ps_t", bufs=2, space="PSUM"))
        ps_s = ctx.enter_context(tc.tile_pool(name="ps_s", bufs=2, space="PSUM"))
        ps_o = ctx.enter_context(tc.tile_pool(name="ps_o", bufs=2, space="PSUM"))
        ps_p = ctx.enter_context(tc.tile_pool(name="ps_p", bufs=1, space="PSUM"))

        ident = consts.tile([128, 128], BF16, tag="ident")
        nc.sync.dma_start(ident[:, :], ident_d[:, :])
        mask = consts.tile([128, 128], BF16, tag="mask")
        nc.sync.dma_start(mask[:, :], mask_d[:, :])

        def staged_load(tag, dram, cols, eng=None):
            """[T-ish, cols] DRAM -> [128, NCH*cols] SBUF (chunk c at col block c)."""
            t_ = consts.tile([128, NCH * cols], BF16, tag=tag, name=tag)
            (eng or nc.sync).dma_start(
                t_[:, :].rearrange("p (c n) -> p c n", c=NCH),
                dram.rearrange("(c p) n -> p c n", p=128))
            return t_

        ctab = staged_load("ctab", c_d[:, :], N)
        stab = staged_load("stab", s_d[:, :], N)
        qsb, vsb, osb = [], [], []
        for h in range(HPC):
            rows = slice(h * T, (h + 1) * T)
            qsb.append(staged_load(f"q{h}", q_d[rows, :], N))
            vsb.append(staged_load(f"v{h}", v_d[rows, :], DV))
            osb.append(consts.tile([128, NCH * DV], BF16, tag=f"o{h}", name=f"osb{h}"))

        p_sb = [None] * HPC

        for hp in range(HPC // 2):
          # Two heads interleaved per pass; per-head P accumulators in PSUM
          p_ps_pair = [
              ps_p.tile([128, 2 * DV], F32, tag=f"pps{k}", name=f"pps{k}_{hp}")
              for k in range(2)
          ]
          for i in range(NCH):
            for k in range(2):
                h = hp * 2 + k
                first = i == 0
                last = i == NCH - 1
                qi = qsb[h][:, i * N:(i + 1) * N]
                vi = vsb[h][:, i * DV:(i + 1) * DV]
                ci = ctab[:, i * N:(i + 1) * N]
                si = stab[:, i * N:(i + 1) * N]

                # RoPE halves: A = q*cos (DVE), B = qsw*ssin (GPSIMD)
                a_t = rope.tile([CH, N], BF16, tag="ra")
                nc.gpsimd.tensor_mul(a_t[:, :], qi, ci)
                b_t = rope.tile([CH, N], BF16, tag="rb")
                q_sw = qi.rearrange("p (a b) -> p a b", b=2)[:, :, ::-1]
                nc.gpsimd.tensor_mul(
                    b_t[:, :].rearrange("p (a b) -> p a b", b=2), q_sw,
                    si.rearrange("p (a b) -> p a b", b=2))

                # QRT = transpose(A) + transpose(B), accumulated in PSUM
                qrt_ps = ps_t.tile([128, 256], BF16, tag="qrt_ps")
                for half, sl in ((0, slice(0, 128)), (1, slice(128, 256))):
                    nc.tensor.matmul(qrt_ps[:, sl], lhsT=a_t[:, sl], rhs=ident[:, :],
                                     is_transpose=True, start=True, stop=False)
                    nc.tensor.matmul(qrt_ps[:, sl], lhsT=b_t[:, sl], rhs=ident[:, :],
                                     is_transpose=True, start=False, stop=True)
                qrt = qrtp.tile([128, 256], BF16, tag="qrt")
                if i % 4 == 3:
                    nc.vector.tensor_copy(qrt[:, :], qrt_ps[:, :])
                else:
                    nc.scalar.copy(qrt[:, :], qrt_ps[:, :])

                # Intra-chunk scores ST[s, tq] = sum_n QRT[n,s] QRT[n,tq]
                st_ps = ps_s.tile([128, 128], F32, tag="st_ps")
                nc.tensor.matmul(st_ps[:, :], lhsT=qrt[:, 0:128], rhs=qrt[:, 0:128],
                                 start=True, stop=False)
                nc.tensor.matmul(st_ps[:, :], lhsT=qrt[:, 128:256], rhs=qrt[:, 128:256],
                                 start=False, stop=True)
                st_sb = stp.tile([128, 128], BF16, tag="st_sb")
                nc.vector.tensor_mul(st_sb[:, :], st_ps[:, :], mask[:, :])

                # out_i = ST^T @ V (intra) + QR_i @ P_prev (inter)
                o_ps = ps_o.tile([128, DV], F32, tag="o_ps")
                nc.tensor.matmul(o_ps[:, :], lhsT=st_sb[:, :], rhs=vi,
                                 start=True, stop=first)
                if not first:
                    pv = p_sb[h]
                    nc.tensor.matmul(o_ps[:, :], lhsT=qrt[:, 0:128], rhs=pv[:, 0:DV],
                                     start=False, stop=False, skip_group_check=True)
                    nc.tensor.matmul(o_ps[:, :], lhsT=qrt[:, 128:256],
                                     rhs=pv[:, DV:2 * DV],
                                     start=False, stop=True, skip_group_check=True)
                if i % 2 == 0:
                    nc.vector.tensor_copy(osb[h][:, i * DV:(i + 1) * DV], o_ps[:, :])
                else:
                    nc.scalar.copy(osb[h][:, i * DV:(i + 1) * DV], o_ps[:, :])

                # P += QR_i^T @ V_i  (A- and B- contributions, accumulate in PSUM)
                for lo, sl in ((0, slice(0, 128)), (1, slice(128, 256))):
                    reg = p_ps_pair[k][:, lo * DV:(lo + 1) * DV]
                    nc.tensor.matmul(reg, lhsT=a_t[:, sl], rhs=vi,
                                     start=first, stop=False, skip_group_check=True)
                    nc.tensor.matmul(reg, lhsT=b_t[:, sl], rhs=vi,
                                     start=False, stop=last, skip_group_check=True)
                if not last:
                    p_new = pp.tile([128, 2 * DV], BF16, tag="p")
                    if i % 2 == 0:
                        nc.vector.tensor_copy(p_new[:, :], p_ps_pair[k][:, :])
                    else:
                        nc.scalar.copy(p_new[:, :], p_ps_pair[k][:, :])
                    p_sb[h] = p_new
                if i == NCH // 2 - 1 or last:
                    hw = NCH // 2
                    blk = slice(0, hw * DV) if i < hw else slice(hw * DV, NCH * DV)
                    rows_half = slice(h * T + (0 if i < hw else T // 2),
                                      h * T + (T // 2 if i < hw else T))
                    nc.sync.dma_start(
                        o_d[rows_half, :].rearrange("(c p) n -> p c n", p=128),
                        osb[h][:, blk].rearrange("p (c n) -> p c n", c=hw))



    nc.finalize()
    return nc


_NC = None


def _get_nc():
    global _NC
    if _NC is None:
        _NC = _build_nc()
    return _NC


def _host_tables(freqs):
    """cos/sin tables [T, N] from freqs [1,1,1,N] (shared across heads)."""
    f = np.asarray(freqs, dtype=np.float32).reshape(N)
    t = np.arange(T, dtype=np.float32).reshape(T, 1)
    ang = np.mod(t * f.reshape(1, N), 1.0).astype(np.float32) * np.float32(2.0 * math.pi)
    cos = np.cos(ang).astype(np.float32)
    sin = np.sin(ang).astype(np.float32)
    # signed sin: QR[2i] = q[2i]*cos[2i] - q[2i+1]*sin[2i]
    #             QR[2i+1] = q[2i+1]*cos[2i+1] + q[2i]*sin[2i+1]
    ssin = sin.copy()
    ssin[:, 0::2] *= -1.0
    return cos, ssin


def _run(inputs, trace=False, trace_kwargs=None):
    Q = np.ascontiguousarray(np.asarray(inputs["Q"], dtype=np.float32))
    V = np.ascontiguousarray(np.asarray(inputs["V"], dtype=np.float32))
    cos, ssin = _host_tables(inputs["freqs"])

    Qf = Q.reshape(BH, T, N)
    Vf = V.reshape(BH, T, DV)

    q_b = Qf.astype(NPBF16)
    v_b = Vf.astype(NPBF16)
    c_b = cos.astype(NPBF16)
    s_b = ssin.astype(NPBF16)

    in_maps = []
    for c in range(NCORES):
        hs = slice(c * HPC, (c + 1) * HPC)
        in_maps.append({
            "q": np.ascontiguousarray(q_b[hs].reshape(TH, N)),
            "v": np.ascontiguousarray(v_b[hs].reshape(TH, DV)),
            "ctab": c_b,
            "stab": s_b,
        })

    nc = _get_nc()
    kw = {}
    if trace:
        kw = dict(trace=True, trace_kwargs=trace_kwargs or {})
    res = run_bass_kernel_spmd(nc, in_maps, core_ids=list(range(NCORES)), **kw)

    out = np.empty((BH, T, DV), dtype=np.float32)
    for c in range(NCORES):
        out[c * HPC:(c + 1) * HPC] = res.results[c]["out"].reshape(HPC, T, DV)
    return out.reshape(B, NH, T, DV), res


def kernel(**inputs):
    out, _ = _run(inputs, trace=False)
    return out
